# revision 2
# baseline (speedup 1.0000x reference)
"""Trainium2 Bass kernel for nn_BinaryLinear (8-core SPMD) — v3.

z = x @ binarize(W).T + binarize(b); out = relu((z - mean)/(std + eps))

Strategy (data-parallel over the 8192-token batch, 1024 rows/core):
  - DoubleRow fp8 matmuls: x is split into hi+lo e4m3 halves packed as
    adjacent k-subtile pairs (K=256/MM); the binary weight (fp8-exact) is
    broadcast across the pair with a stride-0 AP. HW-probed: DR MM runs at
    the same 216 ns as a bf16 MM but carries 2x contraction -> 2048 MMs
    for the full K=4096 hi+lo product (442 us/core floor).
  - z never round-trips DRAM: PSUM evictions add the bias and write bf16
    z rows held in SBUF; the per-row normalize reads them at the last j.
  - Single read of the f32 weight slice feeds both the mean partial sums
    (AllGather of 8 scalars, cheaper floor than AllReduce) and the bf16
    cast used by the PE transposes.
  - Transpose-then-binarize: PE transposes the bf16 weight slice before
    the global threshold arrives; the PSUM eviction IS the binarize (DVE
    is_gt) writing fp8 directly into the AG staging tile.
  - One early AllGather (2MB/rank) of the transposed binary weight; the
    j loop streams the gathered blocks with double-buffered 2MB loads.
"""
import numpy as np

import concourse.bass as bass
import concourse.mybir as mybir
import concourse.tile as tile
from concourse import bacc
from concourse.bass_utils import run_bass_kernel_spmd
from concourse.masks import make_identity

N_CORES = 8
T_FULL = 8192
D_IN = 4096
D_OUT = 4096
T_SHARD = T_FULL // N_CORES    # 1024
O_SHARD = D_OUT // N_CORES     # 512
P = 128
NK = D_IN // P                 # 32 k-tiles
NM = T_SHARD // P              # 8 token tiles
NJ = D_OUT // O_SHARD          # 8 o-blocks
QW = D_IN // 4                 # 1024: x/w load chunk width
EPS = 1e-5
F32 = mybir.dt.float32
BF16 = mybir.dt.bfloat16
FP8 = mybir.dt.float8e4
DR = mybir.MatmulPerfMode.DoubleRow

_cache: dict = {}
last_exec_time_ns = None


def _pair0(ap2d):
    """View a [128, N] AP as a [128, 2, N] DoubleRow moving operand whose
    pair dim has stride 0 (both pair slots read the same N columns)."""
    return bass.AP(tensor=ap2d.tensor, offset=ap2d.offset,
                   ap=[ap2d.ap[0], [0, 2], ap2d.ap[-1]])


def _bcast(ap, n_part):
    return bass.AP(tensor=ap.tensor, offset=ap.offset, ap=[[0, n_part], [1, 1]])


def _build():
    nc = bacc.Bacc("TRN2", target_bir_lowering=False, debug=False,
                   num_devices=N_CORES)
    x_in = nc.dram_tensor("x", [T_SHARD, D_IN], F32, kind="ExternalInput")
    w_in = nc.dram_tensor("w", [O_SHARD, D_IN], F32, kind="ExternalInput")
    b_in = nc.dram_tensor("b", [D_OUT], F32, kind="ExternalInput")
    out_ext = nc.dram_tensor("out", [T_SHARD, D_OUT], F32, kind="ExternalOutput")

    with tile.TileContext(nc) as tc:
        with (
            tc.tile_pool(name="xT_pool", bufs=1) as xT_pool,
            tc.tile_pool(name="z_pool", bufs=1) as z_pool,
            tc.tile_pool(name="wg_pool", bufs=2) as wg_pool,
            tc.tile_pool(name="wstage", bufs=2) as wstage,
            tc.tile_pool(name="xstage", bufs=2) as xstage,
            tc.tile_pool(name="wbf_pool", bufs=3) as wbf_pool,
            tc.tile_pool(name="wqb_pool", bufs=2) as wqb_pool,
            tc.tile_pool(name="xbf_pool", bufs=2) as xbf_pool,
            tc.tile_pool(name="out_pool", bufs=2) as out_pool,
            tc.tile_pool(name="bias_pool", bufs=1) as bias_pool,
            tc.tile_pool(name="small", bufs=1) as small,
            tc.tile_pool(name="psum", bufs=3, space="PSUM") as psum,
            tc.tile_pool(name="ptr_w", bufs=2, space="PSUM") as ptr_w,
            tc.tile_pool(name="ptr_x", bufs=3, space="PSUM") as ptr_x,
            tc.tile_pool(name="dram", bufs=1, space="DRAM") as dram,
        ):
            # ---- resident tiles ----
            # xT pairs: ksub 2k = hi(e4m3), 2k+1 = lo; cols m*128..(m+1)*128
            xT = xT_pool.tile([P, 2 * NK, T_SHARD], FP8)               # 8 MB
            z_sb = [z_pool.tile([P, D_OUT], BF16, name=f"z{m}")        # 8 MB
                    for m in range(NM)]

            identity = small.tile([P, P], BF16)
            make_identity(nc, identity)

            # ---- A: w pass-1: load f32 chunks, partial sums for threshold ----
            colsums = small.tile([P, 16], F32)
            w_first_dma = None
            for ch in range(16):
                c4, q = ch // 4, ch % 4
                wf = wstage.tile([P, QW], F32, name=f"wf{ch}", tag="ws")
                wdma = nc.sync.dma_start(
                    out=wf[:],
                    in_=w_in[c4 * P:(c4 + 1) * P, q * QW:(q + 1) * QW])
                if w_first_dma is None:
                    w_first_dma = wdma
                nc.vector.reduce_sum(colsums[:, ch:ch + 1], wf[:],
                                     axis=mybir.AxisListType.X)
            rowsum = small.tile([P, 1], F32)
            nc.vector.reduce_sum(rowsum[:], colsums[:], axis=mybir.AxisListType.X)
            # partition-sum via DRAM bounce (keeps PE out of the thr chain)
            rs_dram = dram.tile([P], F32)
            nc.gpsimd.dma_start(
                out=rs_dram[:].rearrange("(p o) -> p o", p=P), in_=rowsum[:])
            rs_row = small.tile([1, P], F32)
            nc.gpsimd.dma_start(
                out=rs_row[:], in_=rs_dram[:].rearrange("(o p) -> o p", o=1))
            ar_sb = small.tile([1, 1], F32)
            nc.vector.reduce_sum(ar_sb[:], rs_row[:], axis=mybir.AxisListType.X)
            ag_sum_in = dram.tile([1], F32)
            ag_sum_out = dram.tile([N_CORES], F32, addr_space="Shared")
            nc.gpsimd.dma_start(out=ag_sum_in[:].rearrange("(o d) -> o d", o=1),
                                in_=ar_sb[:])
            nc.gpsimd.collective_compute(
                "AllGather", mybir.AluOpType.bypass,
                replica_groups=[list(range(N_CORES))],
                ins=[ag_sum_in.opt()], outs=[ag_sum_out.opt()],
            )
            sums8 = small.tile([1, N_CORES], F32)
            nc.gpsimd.dma_start(
                out=sums8[:],
                in_=ag_sum_out[:].rearrange("(o c) -> o c", o=1))
            tot = small.tile([1, 1], F32)
            nc.vector.reduce_sum(tot[:], sums8[:], axis=mybir.AxisListType.X)
            thr1 = small.tile([1, 1], F32)
            nc.vector.tensor_scalar_mul(thr1[:], tot[:], 1.0 / (D_OUT * D_IN))
            # broadcast threshold to 128 partitions via DRAM bounce
            thr_dram = dram.tile([1], F32)
            nc.gpsimd.dma_start(out=thr_dram[:].rearrange("(o d) -> o d", o=1),
                                in_=thr1[:])
            thr_sb = small.tile([P, 1], F32)
            nc.gpsimd.dma_start(out=thr_sb[:], in_=_bcast(thr_dram.opt(), P))

            # ---- B: w pass-2: reload, cast, binarize (gated), transpose ----
            wtr = wg_pool.tile([P, NK, O_SHARD], FP8, name="wtr", tag="wg")
            for c4 in range(4):
                for h in range(2):
                    wbf = wbf_pool.tile([P, 2 * QW], BF16,
                                        name=f"wbf{c4}_{h}", tag="wbf")
                    for qq in range(2):
                        q = h * 2 + qq
                        wf = wstage.tile([P, QW], F32, name=f"wf2_{c4}_{q}",
                                         tag="ws")
                        nc.sync.dma_start(
                            out=wf[:],
                            in_=w_in[c4 * P:(c4 + 1) * P, q * QW:(q + 1) * QW])
                        nc.scalar.copy(out=wbf[:, qq * QW:(qq + 1) * QW],
                                       in_=wf[:])
                    wq = wqb_pool.tile([P, 2 * QW], BF16,
                                       name=f"wq{c4}_{h}", tag="wqb")
                    nc.vector.tensor_scalar(
                        out=wq[:], in0=wbf[:],
                        scalar1=thr_sb[:], scalar2=None,
                        op0=mybir.AluOpType.is_gt,
                    )
                    for kk in range(2 * QW // P):
                        k = h * (2 * QW // P) + kk
                        pt = ptr_w.tile([P, P], BF16, name=f"wpt{c4}_{k}",
                                        tag="ptrw")
                        nc.tensor.transpose(pt[:], wq[:, kk * P:(kk + 1) * P],
                                            identity[:])
                        nc.vector.tensor_copy(
                            out=wtr[:, k, c4 * P:(c4 + 1) * P], in_=pt[:])

            # ---- C: AllGather the transposed binary slice (2MB/rank) ----
            ag_in = dram.tile([P * NK * O_SHARD], FP8)
            ag_out = dram.tile([N_CORES, P * NK * O_SHARD], FP8,
                               addr_space="Shared")
            nc.sync.dma_start(
                out=ag_in[:].rearrange("(p k o) -> p k o", p=P, k=NK),
                in_=wtr[:])
            nc.gpsimd.collective_compute(
                "AllGather", mybir.AluOpType.bypass,
                replica_groups=[list(range(N_CORES))],
                ins=[ag_in.opt()], outs=[ag_out.opt()],
            )

            # ---- D: bias binarize ([128, 32] layout) + DMA broadcast ----
            bsb = small.tile([P, D_OUT // P], F32)   # b[p*32 + j]
            nc.scalar.dma_start(
                out=bsb[:], in_=b_in[:].rearrange("(p j) -> p j", p=P))
            bpart = small.tile([P, 1], F32)
            nc.vector.reduce_sum(bpart[:], bsb[:], axis=mybir.AxisListType.X)
            bp_dram = dram.tile([P], F32)
            nc.gpsimd.dma_start(
                out=bp_dram[:].rearrange("(p o) -> p o", p=P), in_=bpart[:])
            bp_row = small.tile([1, P], F32)
            nc.gpsimd.dma_start(
                out=bp_row[:], in_=bp_dram[:].rearrange("(o p) -> o p", o=1))
            b_mean1 = small.tile([1, 1], F32)
            nc.vector.reduce_sum(b_mean1[:], bp_row[:], axis=mybir.AxisListType.X)
            nc.vector.tensor_scalar_mul(b_mean1[:], b_mean1[:], 1.0 / D_OUT)
            bm_dram = dram.tile([1], F32)
            nc.gpsimd.dma_start(out=bm_dram[:].rearrange("(o d) -> o d", o=1),
                                in_=b_mean1[:])
            b_mean = small.tile([P, 1], F32)
            nc.gpsimd.dma_start(out=b_mean[:], in_=_bcast(bm_dram.opt(), P))
            b_q128 = small.tile([P, D_OUT // P], FP8)
            nc.vector.tensor_scalar(
                out=b_q128[:], in0=bsb[:], scalar1=b_mean[:], scalar2=None,
                op0=mybir.AluOpType.is_gt,
            )
            bq_dram = dram.tile([D_OUT], FP8)
            nc.gpsimd.dma_start(
                out=bq_dram[:].rearrange("(p j) -> p j", p=P), in_=b_q128[:])
            bias_bcast = bias_pool.tile([P, D_OUT], FP8)                # 0.5 MB
            nc.gpsimd.dma_start(
                out=bias_bcast[:],
                in_=bass.AP(tensor=bq_dram.opt().tensor,
                            offset=bq_dram.opt().offset,
                            ap=[[0, P], [1, D_OUT]]))

            # ---- E: x load/cast/transpose -> hi/lo pair split ----
            xq_first = None
            XC = 512
            for mx in range(NM):
                for q in range(D_IN // XC):
                    xf = xstage.tile([P, XC], F32, name=f"xf{mx}_{q}", tag="xs")
                    xd = nc.scalar.dma_start(
                        out=xf[:],
                        in_=x_in[mx * P:(mx + 1) * P, q * XC:(q + 1) * XC])
                    if xq_first is None:
                        xq_first = xd
                        tile.add_dep_helper(
                            w_first_dma.ins, xd.ins, sync=True,
                            reason="w loads win HBM first")
                    xbf = xbf_pool.tile([P, XC], BF16, name=f"xbf{mx}_{q}",
                                        tag="xbf")
                    nc.scalar.copy(out=xbf[:], in_=xf[:])
                    for kk in range(XC // P):
                        k = q * (XC // P) + kk
                        pt = ptr_x.tile([P, P], BF16, name=f"xpt{mx}_{k}",
                                        tag="ptrx")
                        nc.tensor.transpose(pt[:], xbf[:, kk * P:(kk + 1) * P],
                                            identity[:])
                        hi = xT[:, 2 * k, mx * P:(mx + 1) * P]
                        lo = xT[:, 2 * k + 1, mx * P:(mx + 1) * P]
                        nc.scalar.copy(out=hi, in_=pt[:])
                        nc.vector.tensor_tensor(
                            out=lo, in0=pt[:], in1=hi,
                            op=mybir.AluOpType.subtract)

            # ---- F: main loop (fixed j order; all blocks from AG out) ----
            stats = [small.tile([P, NJ, 6], F32, name=f"stats{m}")
                     for m in range(NM)]
            for jj in range(NJ):
                wg = wg_pool.tile([P, NK, O_SHARD], FP8, name=f"wg{jj}",
                                  tag="wg")
                nc.sync.dma_start(
                    out=wg[:],
                    in_=ag_out[jj].rearrange("(p k o) -> p k o", p=P, k=NK))
                for m in range(NM):
                    ps = psum.tile([P, O_SHARD], F32, name=f"ps{jj}_{m}",
                                   tag="ps")
                    for k in range(NK):
                        nc.tensor.matmul(
                            ps[:],
                            xT[:, 2 * k:2 * k + 2, m * P:(m + 1) * P],
                            _pair0(wg[:, k, :]),
                            start=(k == 0), stop=(k == NK - 1),
                            perf_mode=DR)
                    zrow = z_sb[m]
                    nc.vector.tensor_tensor(
                        out=zrow[:, jj * O_SHARD:(jj + 1) * O_SHARD],
                        in0=ps[:],
                        in1=bias_bcast[:, jj * O_SHARD:(jj + 1) * O_SHARD],
                        op=mybir.AluOpType.add)
                    nc.vector.bn_stats(
                        out=stats[m][:, jj, :],
                        in_=zrow[:, jj * O_SHARD:(jj + 1) * O_SHARD])
                    if jj == NJ - 1:
                        mv = small.tile([P, 2], F32, name=f"mv{m}")
                        nc.vector.bn_aggr(out=mv[:], in_=stats[m][:])
                        std = small.tile([P, 1], F32, name=f"std{m}")
                        nc.scalar.sqrt(std[:], mv[:, 1:2])
                        nc.vector.tensor_scalar_add(std[:], std[:], EPS)
                        rstd = small.tile([P, 1], F32, name=f"rstd{m}")
                        nc.vector.reciprocal(rstd[:], std[:])
                        shift = small.tile([P, 1], F32, name=f"shift{m}")
                        nc.vector.tensor_mul(shift[:], mv[:, 0:1], rstd[:])
                        nc.vector.tensor_scalar_mul(shift[:], shift[:], -1.0)
                        for q in range(NJ):
                            ot = out_pool.tile([P, O_SHARD], F32,
                                               name=f"ot{m}_{q}", tag="ot")
                            nc.scalar.activation(
                                out=ot[:],
                                in_=zrow[:, q * O_SHARD:(q + 1) * O_SHARD],
                                func=mybir.ActivationFunctionType.Relu,
                                bias=shift[:], scale=rstd[:],
                            )
                            nc.scalar.dma_start(
                                out=out_ext[m * P:(m + 1) * P,
                                            q * O_SHARD:(q + 1) * O_SHARD],
                                in_=ot[:])

    nc.finalize()
    return nc


def kernel(x: np.ndarray, weight: np.ndarray, b: np.ndarray) -> np.ndarray:
    global last_exec_time_ns
    import os
    x = np.ascontiguousarray(x, dtype=np.float32)
    weight = np.ascontiguousarray(weight, dtype=np.float32)
    b = np.ascontiguousarray(b, dtype=np.float32)
    assert x.shape == (T_FULL, D_IN) and weight.shape == (D_OUT, D_IN)

    if "nc" not in _cache:
        _cache["nc"] = _build()
    nc = _cache["nc"]

    in_maps = [
        {
            "x": x[c * T_SHARD:(c + 1) * T_SHARD],
            "w": weight[c * O_SHARD:(c + 1) * O_SHARD],
            "b": b,
        }
        for c in range(N_CORES)
    ]
    trace = os.environ.get("BASS_KERNEL_TRACE", "") == "1"
    res = run_bass_kernel_spmd(nc, in_maps, list(range(N_CORES)), trace=trace)
    last_exec_time_ns = res.exec_time_ns
    return np.concatenate([res.results[c]["out"] for c in range(N_CORES)],
                          axis=0)


# revision 3
# speedup vs baseline: 1.0426x; 1.0426x over previous
"""Trainium2 Bass kernel for nn_BinaryLinear (8-core SPMD) — v4.

z = x @ binarize(W).T + binarize(b); out = relu((z - mean)/(std + eps))

v4 over v3 (see v3 docstring for the DoubleRow hi/lo scheme):
  - Own-slice-first main loop: iteration jj processes o-block
    r = (pid + jj) & 7. jj=0 is the core's own block, fed straight from
    the resident transposed tile — it runs before/while the AllGather
    flies. z column-slots are permuted per core (layernorm stats are
    permutation-invariant); only the wg-load sources, the permuted bias
    loads, and the final out-store column offsets use register offsets.
  - AllGather split into 2x 1MB k-half stages (2MB single AG measured
    140us vs 51us per 1MB); main loop split into pass-A (k0..15, biased
    partial z in bf16) and pass-B (k16..31 added on top + stats), so
    pass-A hides AG1.
  - Batched transpose evictions: 4 transposes land in one [128,512] PSUM
    tile; one ACT copy (hi) + one DVE subtract (lo) — or one is_gt for
    the w side — evict 4 k-tiles at once (v3's per-tile evictions made
    the x pipeline latency-bound).
  - The w pass-1 mean-reduces are fenced before the x-path's first DVE
    op so the threshold collective triggers at ~30us.
"""
import numpy as np

import concourse.bass as bass
import concourse.mybir as mybir
import concourse.tile as tile
from concourse import bacc
from concourse.bass_utils import run_bass_kernel_spmd
from concourse.masks import make_identity

N_CORES = 8
T_FULL = 8192
D_IN = 4096
D_OUT = 4096
T_SHARD = T_FULL // N_CORES    # 1024
O_SHARD = D_OUT // N_CORES     # 512
P = 128
NK = D_IN // P                 # 32 k-tiles
NKH = NK // 2                  # 16 per AG stage
NM = T_SHARD // P              # 8 token tiles
NJ = D_OUT // O_SHARD          # 8 o-blocks
QW = D_IN // 4                 # 1024
EPS = 1e-5
F32 = mybir.dt.float32
BF16 = mybir.dt.bfloat16
FP8 = mybir.dt.float8e4
DR = mybir.MatmulPerfMode.DoubleRow

_cache: dict = {}
last_exec_time_ns = None


def _pair0(ap2d):
    return bass.AP(tensor=ap2d.tensor, offset=ap2d.offset,
                   ap=[ap2d.ap[0], [0, 2], ap2d.ap[-1]])


def _bcast(ap, n_part, width=1):
    return bass.AP(tensor=ap.tensor, offset=ap.offset,
                   ap=[[0, n_part], [1, width]])


def _build():
    nc = bacc.Bacc("TRN2", target_bir_lowering=False, debug=False,
                   num_devices=N_CORES)
    x_in = nc.dram_tensor("x", [T_SHARD, D_IN], F32, kind="ExternalInput")
    w_in = nc.dram_tensor("w", [O_SHARD, D_IN], F32, kind="ExternalInput")
    b_in = nc.dram_tensor("b", [D_OUT], F32, kind="ExternalInput")
    out_ext = nc.dram_tensor("out", [T_SHARD, D_OUT], F32, kind="ExternalOutput")

    with tile.TileContext(nc) as tc:
        with (
            tc.tile_pool(name="xT_pool", bufs=1) as xT_pool,
            tc.tile_pool(name="z_pool", bufs=1) as z_pool,
            tc.tile_pool(name="wtr_pool", bufs=1) as wtr_pool,
            tc.tile_pool(name="wg_pool", bufs=3) as wg_pool,
            tc.tile_pool(name="wstage", bufs=2) as wstage,
            tc.tile_pool(name="xstage", bufs=2) as xstage,
            tc.tile_pool(name="wbf_pool", bufs=2) as wbf_pool,
            tc.tile_pool(name="wqb_pool", bufs=1) as wqb_pool,
            tc.tile_pool(name="xbf_pool", bufs=2) as xbf_pool,
            tc.tile_pool(name="out_pool", bufs=2) as out_pool,
            tc.tile_pool(name="bias_pool", bufs=1) as bias_pool,
            tc.tile_pool(name="small", bufs=1) as small,
            tc.tile_pool(name="psum", bufs=3, space="PSUM") as psum,
            tc.tile_pool(name="ptr_w", bufs=2, space="PSUM") as ptr_w,
            tc.tile_pool(name="ptr_x", bufs=2, space="PSUM") as ptr_x,
            tc.tile_pool(name="dram", bufs=1, space="DRAM") as dram,
        ):
            # xT pairs: ksub 2k = hi(e4m3), 2k+1 = lo; cols m*128..(m+1)*128
            xT = xT_pool.tile([P, 2 * NK, T_SHARD], FP8)               # 8 MB
            z_sb = [z_pool.tile([P, D_OUT], BF16, name=f"z{m}")        # 8 MB
                    for m in range(NM)]
            wtr = wtr_pool.tile([P, NK, O_SHARD], FP8)                 # 2 MB

            identity = small.tile([P, P], BF16)
            make_identity(nc, identity)

            # ---- A: w pass-1: load f32 chunks, partial sums -> threshold ----
            colsums = small.tile([P, 16], F32)
            w_first_dma = None
            last_reduce = None
            for ch in range(16):
                c4, q = ch // 4, ch % 4
                wf = wstage.tile([P, QW], F32, name=f"wf{ch}", tag="ws")
                wdma = nc.sync.dma_start(
                    out=wf[:],
                    in_=w_in[c4 * P:(c4 + 1) * P, q * QW:(q + 1) * QW])
                if w_first_dma is None:
                    w_first_dma = wdma
                last_reduce = nc.vector.reduce_sum(
                    colsums[:, ch:ch + 1], wf[:], axis=mybir.AxisListType.X)
            rowsum = small.tile([P, 1], F32)
            nc.vector.reduce_sum(rowsum[:], colsums[:], axis=mybir.AxisListType.X)
            rs_dram = dram.tile([P], F32)
            nc.gpsimd.dma_start(
                out=rs_dram[:].rearrange("(p o) -> p o", p=P), in_=rowsum[:])
            rs_row = small.tile([1, P], F32)
            nc.gpsimd.dma_start(
                out=rs_row[:], in_=rs_dram[:].rearrange("(o p) -> o p", o=1))
            ar_sb = small.tile([1, 1], F32)
            nc.vector.reduce_sum(ar_sb[:], rs_row[:], axis=mybir.AxisListType.X)
            ag_sum_in = dram.tile([1], F32)
            ag_sum_out = dram.tile([N_CORES], F32, addr_space="Shared")
            nc.gpsimd.dma_start(out=ag_sum_in[:].rearrange("(o d) -> o d", o=1),
                                in_=ar_sb[:])
            nc.gpsimd.collective_compute(
                "AllGather", mybir.AluOpType.bypass,
                replica_groups=[list(range(N_CORES))],
                ins=[ag_sum_in.opt()], outs=[ag_sum_out.opt()],
            )
            sums8 = small.tile([1, N_CORES], F32)
            nc.gpsimd.dma_start(
                out=sums8[:],
                in_=ag_sum_out[:].rearrange("(o c) -> o c", o=1))
            tot = small.tile([1, 1], F32)
            nc.vector.reduce_sum(tot[:], sums8[:], axis=mybir.AxisListType.X)
            thr1 = small.tile([1, 1], F32)
            nc.vector.tensor_scalar_mul(thr1[:], tot[:], 1.0 / (D_OUT * D_IN))
            thr_dram = dram.tile([1], F32)
            nc.gpsimd.dma_start(out=thr_dram[:].rearrange("(o d) -> o d", o=1),
                                in_=thr1[:])
            thr_sb = small.tile([P, 1], F32)
            nc.gpsimd.dma_start(out=thr_sb[:], in_=_bcast(thr_dram.opt(), P))

            # ---- B/C: w pass-2 per k-half: reload, cast, binarize,
            #           transpose (batched evictions), store + AllGather ----
            ag_in = [dram.tile([P * NKH * O_SHARD], FP8, name=f"agi{h}")
                     for h in range(2)]
            ag_out = [dram.tile([N_CORES, P * NKH * O_SHARD], FP8,
                                name=f"ago{h}", addr_space="Shared")
                      for h in range(2)]
            for h in range(2):
                for c4 in range(4):
                    wbf = wbf_pool.tile([P, 2 * QW], BF16,
                                        name=f"wbf{c4}_{h}", tag="wbf")
                    for qq in range(2):
                        q = h * 2 + qq
                        wf = wstage.tile([P, QW], F32, name=f"wf2_{c4}_{q}",
                                         tag="ws")
                        nc.sync.dma_start(
                            out=wf[:],
                            in_=w_in[c4 * P:(c4 + 1) * P, q * QW:(q + 1) * QW])
                        nc.scalar.copy(out=wbf[:, qq * QW:(qq + 1) * QW],
                                       in_=wf[:])
                    wq = wqb_pool.tile([P, 2 * QW], BF16,
                                       name=f"wq{c4}_{h}", tag="wqb")
                    nc.vector.tensor_scalar(
                        out=wq[:], in0=wbf[:], scalar1=thr_sb[:], scalar2=None,
                        op0=mybir.AluOpType.is_gt,
                    )
                    for g in range(4):          # groups of 4 k-tiles
                        pt = ptr_w.tile([P, 4 * P], BF16,
                                        name=f"wpt{h}_{c4}_{g}", tag="ptrw")
                        for i in range(4):
                            nc.tensor.transpose(
                                pt[:, i * P:(i + 1) * P],
                                wq[:, (g * 4 + i) * P:(g * 4 + i + 1) * P],
                                identity[:])
                        k0 = h * NKH + g * 4
                        nc.vector.tensor_copy(
                            out=wtr[:, k0:k0 + 4, c4 * P:(c4 + 1) * P],
                            in_=pt[:])
                nc.sync.dma_start(
                    out=ag_in[h][:].rearrange("(p k o) -> p k o", p=P, k=NKH),
                    in_=wtr[:, h * NKH:(h + 1) * NKH, :])
                nc.gpsimd.collective_compute(
                    "AllGather", mybir.AluOpType.bypass,
                    replica_groups=[list(range(N_CORES))],
                    ins=[ag_in[h].opt()], outs=[ag_out[h].opt()],
                )

            # ---- D: bias binarize ([128, 32] layout) + permuted broadcast ----
            bsb = small.tile([P, D_OUT // P], F32)
            nc.scalar.dma_start(
                out=bsb[:], in_=b_in[:].rearrange("(p j) -> p j", p=P))
            bpart = small.tile([P, 1], F32)
            nc.vector.reduce_sum(bpart[:], bsb[:], axis=mybir.AxisListType.X)
            bp_dram = dram.tile([P], F32)
            nc.gpsimd.dma_start(
                out=bp_dram[:].rearrange("(p o) -> p o", p=P), in_=bpart[:])
            bp_row = small.tile([1, P], F32)
            nc.gpsimd.dma_start(
                out=bp_row[:], in_=bp_dram[:].rearrange("(o p) -> o p", o=1))
            b_mean1 = small.tile([1, 1], F32)
            nc.vector.reduce_sum(b_mean1[:], bp_row[:], axis=mybir.AxisListType.X)
            nc.vector.tensor_scalar_mul(b_mean1[:], b_mean1[:], 1.0 / D_OUT)
            bm_dram = dram.tile([1], F32)
            nc.gpsimd.dma_start(out=bm_dram[:].rearrange("(o d) -> o d", o=1),
                                in_=b_mean1[:])
            b_mean = small.tile([P, 1], F32)
            nc.gpsimd.dma_start(out=b_mean[:], in_=_bcast(bm_dram.opt(), P))
            b_q128 = small.tile([P, D_OUT // P], FP8)
            nc.vector.tensor_scalar(
                out=b_q128[:], in0=bsb[:], scalar1=b_mean[:], scalar2=None,
                op0=mybir.AluOpType.is_gt,
            )
            bq_dram = dram.tile([D_OUT], FP8)
            nc.gpsimd.dma_start(
                out=bq_dram[:].rearrange("(p j) -> p j", p=P), in_=b_q128[:])
            # permuted bias: slot jj holds o-block (pid + jj) & 7
            bias_bcast = bias_pool.tile([P, D_OUT], FP8)                # 0.5 MB
            pid_g = nc.gpsimd.partition_id()
            for jj in range(NJ):
                r = (pid_g + jj) & 7
                nc.gpsimd.dma_start(
                    out=bias_bcast[:, jj * O_SHARD:(jj + 1) * O_SHARD],
                    in_=bass.AP(tensor=bq_dram.opt().tensor,
                                offset=r * O_SHARD,
                                ap=[[0, P], [1, O_SHARD]]))

            # ---- E: x load/cast/transpose, batched hi/lo pair split ----
            xq_first = None
            first_lo = None
            XC = 512
            for mx in range(NM):
                for q in range(D_IN // XC):
                    xf = xstage.tile([P, XC], F32, name=f"xf{mx}_{q}", tag="xs")
                    xd = nc.scalar.dma_start(
                        out=xf[:],
                        in_=x_in[mx * P:(mx + 1) * P, q * XC:(q + 1) * XC])
                    if xq_first is None:
                        xq_first = xd
                        tile.add_dep_helper(
                            w_first_dma.ins, xd.ins, sync=True,
                            reason="w loads win HBM first")
                    xbf = xbf_pool.tile([P, XC], BF16, name=f"xbf{mx}_{q}",
                                        tag="xbf")
                    nc.scalar.copy(out=xbf[:], in_=xf[:])
                    # one [128, 512] psum group = 4 transposed k-tiles
                    pt = ptr_x.tile([P, 4 * P], BF16, name=f"xpt{mx}_{q}",
                                    tag="ptrx")
                    for i in range(XC // P):
                        nc.tensor.transpose(pt[:, i * P:(i + 1) * P],
                                            xbf[:, i * P:(i + 1) * P],
                                            identity[:])
                    k0 = q * (XC // P)
                    xap = xT[:]
                    hi = bass.AP(tensor=xap.tensor,
                                 offset=(2 * k0) * T_SHARD + mx * P,
                                 ap=[xap.ap[0], [2 * T_SHARD, 4], [1, P]])
                    lo = bass.AP(tensor=xap.tensor,
                                 offset=(2 * k0 + 1) * T_SHARD + mx * P,
                                 ap=[xap.ap[0], [2 * T_SHARD, 4], [1, P]])
                    nc.scalar.copy(out=hi, in_=pt[:])
                    lo_tt = nc.vector.tensor_tensor(
                        out=lo, in0=pt[:], in1=hi,
                        op=mybir.AluOpType.subtract)
                    if first_lo is None:
                        first_lo = lo_tt
                        tile.add_dep_helper(
                            last_reduce.ins, lo_tt.ins, sync=True,
                            reason="threshold reduces first on DVE")

            # ---- F: main loop; slot jj <-> o-block (pid+jj)&7 ----
            stats = [small.tile([P, NJ, 6], F32, name=f"stats{m}")
                     for m in range(NM)]
            pid_s = nc.sync.partition_id()

            def wg_load(h, jj, tag_name):
                """Load o-block (pid+jj)&7 of AG stage h into SBUF."""
                wg = wg_pool.tile([P, NKH, O_SHARD], FP8, name=tag_name,
                                  tag="wg")
                r = (pid_s + jj) & 7
                src = bass.AP(
                    tensor=ag_out[h].opt().tensor,
                    offset=r * (P * NKH * O_SHARD),
                    ap=[[NKH * O_SHARD, P], [O_SHARD, NKH], [1, O_SHARD]])
                nc.sync.dma_start(out=wg[:], in_=src)
                return wg

            def mm_group(ps, wg_ap_fn, m, ks, start, stop):
                for i, k in enumerate(ks):
                    nc.tensor.matmul(
                        ps[:],
                        xT[:, 2 * k:2 * k + 2, m * P:(m + 1) * P],
                        _pair0(wg_ap_fn(k)),
                        start=(start and i == 0),
                        stop=(stop and i == len(ks) - 1),
                        perf_mode=DR)

            # pass-A: jj=0 full-k from resident wtr; jj>=1 k-half 0
            for jj in range(NJ):
                if jj == 0:
                    wg_ap_fn = lambda k: wtr[:, k, :]
                    ks = list(range(NK))
                else:
                    wg = wg_load(0, jj, f"wgA{jj}")
                    wg_ap_fn = (lambda wg: lambda k: wg[:, k, :])(wg)
                    ks = list(range(NKH))
                for m in range(NM):
                    ps = psum.tile([P, O_SHARD], F32, name=f"psA{jj}_{m}",
                                   tag="ps")
                    mm_group(ps, wg_ap_fn, m, ks, True, True)
                    zrow = z_sb[m]
                    nc.vector.tensor_tensor(
                        out=zrow[:, jj * O_SHARD:(jj + 1) * O_SHARD],
                        in0=ps[:],
                        in1=bias_bcast[:, jj * O_SHARD:(jj + 1) * O_SHARD],
                        op=mybir.AluOpType.add)
                    if jj == 0:
                        nc.vector.bn_stats(
                            out=stats[m][:, 0, :],
                            in_=zrow[:, 0:O_SHARD])

            # pass-B: jj>=1 k-half 1, added onto partial z; stats; normalize
            pid_sc = nc.scalar.partition_id()
            for jj in range(1, NJ):
                wg = wg_load(1, jj, f"wgB{jj}")
                wg_ap_fn = (lambda wg: lambda k: wg[:, k - NKH, :])(wg)
                for m in range(NM):
                    ps = psum.tile([P, O_SHARD], F32, name=f"psB{jj}_{m}",
                                   tag="ps")
                    mm_group(ps, wg_ap_fn, m, list(range(NKH, NK)), True, True)
                    zrow = z_sb[m]
                    nc.vector.tensor_tensor(
                        out=zrow[:, jj * O_SHARD:(jj + 1) * O_SHARD],
                        in0=ps[:],
                        in1=zrow[:, jj * O_SHARD:(jj + 1) * O_SHARD],
                        op=mybir.AluOpType.add)
                    nc.vector.bn_stats(
                        out=stats[m][:, jj, :],
                        in_=zrow[:, jj * O_SHARD:(jj + 1) * O_SHARD])
                    if jj == NJ - 1:
                        mv = small.tile([P, 2], F32, name=f"mv{m}")
                        nc.vector.bn_aggr(out=mv[:], in_=stats[m][:])
                        std = small.tile([P, 1], F32, name=f"std{m}")
                        nc.scalar.sqrt(std[:], mv[:, 1:2])
                        nc.vector.tensor_scalar_add(std[:], std[:], EPS)
                        rstd = small.tile([P, 1], F32, name=f"rstd{m}")
                        nc.vector.reciprocal(rstd[:], std[:])
                        shift = small.tile([P, 1], F32, name=f"shift{m}")
                        nc.vector.tensor_mul(shift[:], mv[:, 0:1], rstd[:])
                        nc.vector.tensor_scalar_mul(shift[:], shift[:], -1.0)
                        for qs in range(NJ):
                            ot = out_pool.tile([P, O_SHARD], F32,
                                               name=f"ot{m}_{qs}", tag="ot")
                            nc.scalar.activation(
                                out=ot[:],
                                in_=zrow[:, qs * O_SHARD:(qs + 1) * O_SHARD],
                                func=mybir.ActivationFunctionType.Relu,
                                bias=shift[:], scale=rstd[:],
                            )
                            rq = (pid_sc + qs) & 7
                            nc.scalar.dma_start(
                                out=out_ext[m * P:(m + 1) * P,
                                            bass.ds(rq * O_SHARD, O_SHARD)],
                                in_=ot[:])

    nc.finalize()
    return nc


def kernel(x: np.ndarray, weight: np.ndarray, b: np.ndarray) -> np.ndarray:
    global last_exec_time_ns
    import os
    x = np.ascontiguousarray(x, dtype=np.float32)
    weight = np.ascontiguousarray(weight, dtype=np.float32)
    b = np.ascontiguousarray(b, dtype=np.float32)
    assert x.shape == (T_FULL, D_IN) and weight.shape == (D_OUT, D_IN)

    if "nc" not in _cache:
        _cache["nc"] = _build()
    nc = _cache["nc"]

    in_maps = [
        {
            "x": x[c * T_SHARD:(c + 1) * T_SHARD],
            "w": weight[c * O_SHARD:(c + 1) * O_SHARD],
            "b": b,
        }
        for c in range(N_CORES)
    ]
    trace = os.environ.get("BASS_KERNEL_TRACE", "") == "1"
    res = run_bass_kernel_spmd(nc, in_maps, list(range(N_CORES)), trace=trace)
    last_exec_time_ns = res.exec_time_ns
    return np.concatenate([res.results[c]["out"] for c in range(N_CORES)],
                          axis=0)


# revision 4
# speedup vs baseline: 1.1126x; 1.0671x over previous
"""Trainium2 Bass kernel for nn_BinaryLinear (8-core SPMD) — v4.

z = x @ binarize(W).T + binarize(b); out = relu((z - mean)/(std + eps))

v4 over v3 (see v3 docstring for the DoubleRow hi/lo scheme):
  - Own-slice-first main loop: iteration jj processes o-block
    r = (pid + jj) & 7. jj=0 is the core's own block, fed straight from
    the resident transposed tile — it runs before/while the AllGather
    flies. z column-slots are permuted per core (layernorm stats are
    permutation-invariant); only the wg-load sources, the permuted bias
    loads, and the final out-store column offsets use register offsets.
  - AllGather split into 2x 1MB k-half stages (2MB single AG measured
    140us vs 51us per 1MB); main loop split into pass-A (k0..15, biased
    partial z in bf16) and pass-B (k16..31 added on top + stats), so
    pass-A hides AG1.
  - Batched transpose evictions: 4 transposes land in one [128,512] PSUM
    tile; one ACT copy (hi) + one DVE subtract (lo) — or one is_gt for
    the w side — evict 4 k-tiles at once (v3's per-tile evictions made
    the x pipeline latency-bound).
  - The w pass-1 mean-reduces are fenced before the x-path's first DVE
    op so the threshold collective triggers at ~30us.
"""
import numpy as np

import concourse.bass as bass
import concourse.mybir as mybir
import concourse.tile as tile
from concourse import bacc
from concourse.bass_utils import run_bass_kernel_spmd
from concourse.masks import make_identity

N_CORES = 8
T_FULL = 8192
D_IN = 4096
D_OUT = 4096
T_SHARD = T_FULL // N_CORES    # 1024
O_SHARD = D_OUT // N_CORES     # 512
P = 128
NK = D_IN // P                 # 32 k-tiles
NKH = NK // 2                  # 16 per AG stage
NM = T_SHARD // P              # 8 token tiles
NJ = D_OUT // O_SHARD          # 8 o-blocks
QW = D_IN // 4                 # 1024
EPS = 1e-5
F32 = mybir.dt.float32
BF16 = mybir.dt.bfloat16
FP8 = mybir.dt.float8e4
DR = mybir.MatmulPerfMode.DoubleRow

_cache: dict = {}
last_exec_time_ns = None


def _pair0(ap2d):
    return bass.AP(tensor=ap2d.tensor, offset=ap2d.offset,
                   ap=[ap2d.ap[0], [0, 2], ap2d.ap[-1]])


def _bcast(ap, n_part, width=1):
    return bass.AP(tensor=ap.tensor, offset=ap.offset,
                   ap=[[0, n_part], [1, width]])


def _build():
    nc = bacc.Bacc("TRN2", target_bir_lowering=False, debug=False,
                   num_devices=N_CORES)
    x_in = nc.dram_tensor("x", [T_SHARD, D_IN], F32, kind="ExternalInput")
    w_in = nc.dram_tensor("w", [O_SHARD, D_IN], F32, kind="ExternalInput")
    b_in = nc.dram_tensor("b", [D_OUT], F32, kind="ExternalInput")
    out_ext = nc.dram_tensor("out", [T_SHARD, D_OUT], F32, kind="ExternalOutput")

    with tile.TileContext(nc) as tc:
        with (
            tc.tile_pool(name="xT_pool", bufs=1) as xT_pool,
            tc.tile_pool(name="z_pool", bufs=1) as z_pool,
            tc.tile_pool(name="wtr_pool", bufs=1) as wtr_pool,
            tc.tile_pool(name="wg_pool", bufs=3) as wg_pool,
            tc.tile_pool(name="wstage", bufs=2) as wstage,
            tc.tile_pool(name="xstage", bufs=2) as xstage,
            tc.tile_pool(name="wqb_pool", bufs=2) as wqb_pool,
            tc.tile_pool(name="xbf_pool", bufs=2) as xbf_pool,
            tc.tile_pool(name="out_pool", bufs=2) as out_pool,
            tc.tile_pool(name="bias_pool", bufs=1) as bias_pool,
            tc.tile_pool(name="small", bufs=1) as small,
            tc.tile_pool(name="psum", bufs=3, space="PSUM") as psum,
            tc.tile_pool(name="ptr_w", bufs=2, space="PSUM") as ptr_w,
            tc.tile_pool(name="ptr_x", bufs=2, space="PSUM") as ptr_x,
            tc.tile_pool(name="dram", bufs=1, space="DRAM") as dram,
        ):
            # xT pairs: ksub 2k = hi(e4m3), 2k+1 = lo; cols m*128..(m+1)*128
            xT = xT_pool.tile([P, 2 * NK, T_SHARD], FP8)               # 8 MB
            z_sb = [z_pool.tile([P, D_OUT], BF16, name=f"z{m}")        # 8 MB
                    for m in range(NM)]
            wtr = wtr_pool.tile([P, NK, O_SHARD], FP8)                 # 2 MB

            identity = small.tile([P, P], BF16)
            make_identity(nc, identity)

            # ---- A: w pass-1: load f32 chunks, partial sums -> threshold ----
            colsums = small.tile([P, 16], F32)
            w_first_dma = None
            last_reduce = None
            for ch in range(16):
                c4, q = ch // 4, ch % 4
                wf = wstage.tile([P, QW], F32, name=f"wf{ch}", tag="ws")
                wdma = nc.sync.dma_start(
                    out=wf[:],
                    in_=w_in[c4 * P:(c4 + 1) * P, q * QW:(q + 1) * QW])
                if w_first_dma is None:
                    w_first_dma = wdma
                w_last_p1_dma = wdma
                last_reduce = nc.vector.reduce_sum(
                    colsums[:, ch:ch + 1], wf[:], axis=mybir.AxisListType.X)
            rowsum = small.tile([P, 1], F32)
            nc.vector.reduce_sum(rowsum[:], colsums[:], axis=mybir.AxisListType.X)
            rs_dram = dram.tile([P], F32)
            nc.sync.dma_start(
                out=rs_dram[:].rearrange("(p o) -> p o", p=P), in_=rowsum[:])
            rs_row = small.tile([1, P], F32)
            nc.sync.dma_start(
                out=rs_row[:], in_=rs_dram[:].rearrange("(o p) -> o p", o=1))
            ar_sb = small.tile([1, 8], F32)
            nc.vector.memset(ar_sb[:], 0.0)
            nc.vector.reduce_sum(ar_sb[:, 0:1], rs_row[:],
                                 axis=mybir.AxisListType.X)
            ar_in = dram.tile([8], F32)
            ar_out = dram.tile([8], F32, addr_space="Shared")
            nc.sync.dma_start(out=ar_in[:].rearrange("(o d) -> o d", o=1),
                              in_=ar_sb[:])
            nc.gpsimd.collective_compute(
                "AllReduce", mybir.AluOpType.add,
                replica_groups=[list(range(N_CORES))],
                ins=[ar_in.opt()], outs=[ar_out.opt()],
            )
            thr_sb = small.tile([P, 1], F32)
            nc.gpsimd.dma_start(out=thr_sb[:], in_=_bcast(ar_out.opt(), P))
            nc.vector.tensor_scalar_mul(thr_sb[:], thr_sb[:],
                                        1.0 / (D_OUT * D_IN))

            # ---- B/C: w pass-2 per k-half: reload, cast, binarize,
            #           transpose (batched evictions), store + AllGather ----
            ag_in = [dram.tile([P * NKH * O_SHARD], FP8, name=f"agi{h}")
                     for h in range(2)]
            ag_out = [dram.tile([N_CORES, P * NKH * O_SHARD], FP8,
                                name=f"ago{h}", addr_space="Shared")
                      for h in range(2)]
            for h in range(2):
                for c4 in range(4):
                    wq = wqb_pool.tile([P, 2 * QW], BF16,
                                       name=f"wq{c4}_{h}", tag="wqb")
                    for qq in range(2):
                        q = h * 2 + qq
                        wf = wstage.tile([P, QW], F32, name=f"wf2_{c4}_{q}",
                                         tag="ws")
                        nc.sync.dma_start(
                            out=wf[:],
                            in_=w_in[c4 * P:(c4 + 1) * P, q * QW:(q + 1) * QW])
                        nc.vector.tensor_scalar(
                            out=wq[:, qq * QW:(qq + 1) * QW], in0=wf[:],
                            scalar1=thr_sb[:], scalar2=None,
                            op0=mybir.AluOpType.is_gt,
                        )
                    for g in range(4):          # groups of 4 k-tiles
                        pt = ptr_w.tile([P, 4 * P], BF16,
                                        name=f"wpt{h}_{c4}_{g}", tag="ptrw")
                        for i in range(4):
                            nc.tensor.transpose(
                                pt[:, i * P:(i + 1) * P],
                                wq[:, (g * 4 + i) * P:(g * 4 + i + 1) * P],
                                identity[:])
                        k0 = h * NKH + g * 4
                        nc.vector.tensor_copy(
                            out=wtr[:, k0:k0 + 4, c4 * P:(c4 + 1) * P],
                            in_=pt[:])
                nc.sync.dma_start(
                    out=ag_in[h][:].rearrange("(p k o) -> p k o", p=P, k=NKH),
                    in_=wtr[:, h * NKH:(h + 1) * NKH, :])
                nc.gpsimd.collective_compute(
                    "AllGather", mybir.AluOpType.bypass,
                    replica_groups=[list(range(N_CORES))],
                    ins=[ag_in[h].opt()], outs=[ag_out[h].opt()],
                )

            # ---- D: bias binarize ([128, 32] layout) + permuted broadcast ----
            bsb = small.tile([P, D_OUT // P], F32)
            nc.scalar.dma_start(
                out=bsb[:], in_=b_in[:].rearrange("(p j) -> p j", p=P))
            bpart = small.tile([P, 1], F32)
            nc.vector.reduce_sum(bpart[:], bsb[:], axis=mybir.AxisListType.X)
            bp_dram = dram.tile([P], F32)
            nc.scalar.dma_start(
                out=bp_dram[:].rearrange("(p o) -> p o", p=P), in_=bpart[:])
            bp_row = small.tile([1, P], F32)
            nc.scalar.dma_start(
                out=bp_row[:], in_=bp_dram[:].rearrange("(o p) -> o p", o=1))
            b_mean1 = small.tile([1, 1], F32)
            nc.vector.reduce_sum(b_mean1[:], bp_row[:], axis=mybir.AxisListType.X)
            nc.vector.tensor_scalar_mul(b_mean1[:], b_mean1[:], 1.0 / D_OUT)
            bm_dram = dram.tile([1], F32)
            nc.gpsimd.dma_start(out=bm_dram[:].rearrange("(o d) -> o d", o=1),
                                in_=b_mean1[:])
            b_mean = small.tile([P, 1], F32)
            nc.gpsimd.dma_start(out=b_mean[:], in_=_bcast(bm_dram.opt(), P))
            b_q128 = small.tile([P, D_OUT // P], FP8)
            nc.vector.tensor_scalar(
                out=b_q128[:], in0=bsb[:], scalar1=b_mean[:], scalar2=None,
                op0=mybir.AluOpType.is_gt,
            )
            bq_dram = dram.tile([D_OUT], FP8)
            nc.gpsimd.dma_start(
                out=bq_dram[:].rearrange("(p j) -> p j", p=P), in_=b_q128[:])
            # permuted bias: slot jj holds o-block (pid + jj) & 7
            bias_bcast = bias_pool.tile([P, D_OUT], FP8)                # 0.5 MB
            pid_g = nc.gpsimd.partition_id()
            for jj in range(NJ):
                r = (pid_g + jj) & 7
                nc.gpsimd.dma_start(
                    out=bias_bcast[:, jj * O_SHARD:(jj + 1) * O_SHARD],
                    in_=bass.AP(tensor=bq_dram.opt().tensor,
                                offset=r * O_SHARD,
                                ap=[[0, P], [1, O_SHARD]]))

            # ---- E: x load/cast/transpose, batched hi/lo pair split ----
            xq_first = None
            first_lo = None
            XC = 512
            for mx in range(NM):
                for q in range(D_IN // XC):
                    xf = xstage.tile([P, XC], F32, name=f"xf{mx}_{q}", tag="xs")
                    xd = nc.scalar.dma_start(
                        out=xf[:],
                        in_=x_in[mx * P:(mx + 1) * P, q * XC:(q + 1) * XC])
                    if xq_first is None:
                        xq_first = xd
                        tile.add_dep_helper(
                            w_last_p1_dma.ins, xd.ins, sync=True,
                            reason="w pass-1 loads win HBM first")
                    xbf = xbf_pool.tile([P, XC], BF16, name=f"xbf{mx}_{q}",
                                        tag="xbf")
                    nc.scalar.copy(out=xbf[:], in_=xf[:])
                    # one [128, 512] psum group = 4 transposed k-tiles
                    pt = ptr_x.tile([P, 4 * P], BF16, name=f"xpt{mx}_{q}",
                                    tag="ptrx")
                    for i in range(XC // P):
                        nc.tensor.transpose(pt[:, i * P:(i + 1) * P],
                                            xbf[:, i * P:(i + 1) * P],
                                            identity[:])
                    k0 = q * (XC // P)
                    xap = xT[:]
                    hi = bass.AP(tensor=xap.tensor,
                                 offset=(2 * k0) * T_SHARD + mx * P,
                                 ap=[xap.ap[0], [2 * T_SHARD, 4], [1, P]])
                    lo = bass.AP(tensor=xap.tensor,
                                 offset=(2 * k0 + 1) * T_SHARD + mx * P,
                                 ap=[xap.ap[0], [2 * T_SHARD, 4], [1, P]])
                    nc.scalar.copy(out=hi, in_=pt[:])
                    lo_tt = nc.vector.tensor_tensor(
                        out=lo, in0=pt[:], in1=hi,
                        op=mybir.AluOpType.subtract)
                    if first_lo is None:
                        first_lo = lo_tt
                        tile.add_dep_helper(
                            last_reduce.ins, lo_tt.ins, sync=True,
                            reason="threshold reduces first on DVE")

            # ---- F: main loop; slot jj <-> o-block (pid+jj)&7 ----
            stats = [small.tile([P, NJ, 6], F32, name=f"stats{m}")
                     for m in range(NM)]
            pid_s = nc.sync.partition_id()

            def wg_load(h, jj, tag_name):
                """Load o-block (pid+jj)&7 of AG stage h into SBUF."""
                wg = wg_pool.tile([P, NKH, O_SHARD], FP8, name=tag_name,
                                  tag="wg")
                r = (pid_s + jj) & 7
                src = bass.AP(
                    tensor=ag_out[h].opt().tensor,
                    offset=r * (P * NKH * O_SHARD),
                    ap=[[NKH * O_SHARD, P], [O_SHARD, NKH], [1, O_SHARD]])
                nc.sync.dma_start(out=wg[:], in_=src)
                return wg

            def mm_group(ps, wg_ap_fn, m, ks, start, stop):
                for i, k in enumerate(ks):
                    nc.tensor.matmul(
                        ps[:],
                        xT[:, 2 * k:2 * k + 2, m * P:(m + 1) * P],
                        _pair0(wg_ap_fn(k)),
                        start=(start and i == 0),
                        stop=(stop and i == len(ks) - 1),
                        perf_mode=DR)

            # pass-A: jj=0 full-k from resident wtr; jj>=1 k-half 0
            for jj in range(NJ):
                if jj == 0:
                    wg_ap_fn = lambda k: wtr[:, k, :]
                    ks = list(range(NK))
                else:
                    wg = wg_load(0, jj, f"wgA{jj}")
                    wg_ap_fn = (lambda wg: lambda k: wg[:, k, :])(wg)
                    ks = list(range(NKH))
                for m in range(NM):
                    ps = psum.tile([P, O_SHARD], F32, name=f"psA{jj}_{m}",
                                   tag="ps")
                    mm_group(ps, wg_ap_fn, m, ks, True, True)
                    zrow = z_sb[m]
                    nc.vector.tensor_tensor(
                        out=zrow[:, jj * O_SHARD:(jj + 1) * O_SHARD],
                        in0=ps[:],
                        in1=bias_bcast[:, jj * O_SHARD:(jj + 1) * O_SHARD],
                        op=mybir.AluOpType.add)
                    if jj == 0:
                        nc.vector.bn_stats(
                            out=stats[m][:, 0, :],
                            in_=zrow[:, 0:O_SHARD])

            # pass-B: m-pair outer, jj inner; wg reloaded per pair (1MB
            # loads overlap MMs); normalize fires per pair, overlapping the
            # next pair's matmuls instead of piling up after the last block.
            pid_sc = nc.scalar.partition_id()
            for mp in range(NM // 2):
                for jj in range(1, NJ):
                    wg = wg_load(1, jj, f"wgB{mp}_{jj}")
                    wg_ap_fn = (lambda wg: lambda k: wg[:, k - NKH, :])(wg)
                    for m in (2 * mp, 2 * mp + 1):
                        ps = psum.tile([P, O_SHARD], F32,
                                       name=f"psB{mp}_{jj}_{m}", tag="ps")
                        mm_group(ps, wg_ap_fn, m, list(range(NKH, NK)),
                                 True, True)
                        zrow = z_sb[m]
                        nc.vector.tensor_tensor(
                            out=zrow[:, jj * O_SHARD:(jj + 1) * O_SHARD],
                            in0=ps[:],
                            in1=zrow[:, jj * O_SHARD:(jj + 1) * O_SHARD],
                            op=mybir.AluOpType.add)
                        nc.vector.bn_stats(
                            out=stats[m][:, jj, :],
                            in_=zrow[:, jj * O_SHARD:(jj + 1) * O_SHARD])
                for m in (2 * mp, 2 * mp + 1):
                    zrow = z_sb[m]
                    mv = small.tile([P, 2], F32, name=f"mv{m}")
                    nc.vector.bn_aggr(out=mv[:], in_=stats[m][:])
                    std = small.tile([P, 1], F32, name=f"std{m}")
                    nc.scalar.sqrt(std[:], mv[:, 1:2])
                    nc.vector.tensor_scalar_add(std[:], std[:], EPS)
                    rstd = small.tile([P, 1], F32, name=f"rstd{m}")
                    nc.vector.reciprocal(rstd[:], std[:])
                    shift = small.tile([P, 1], F32, name=f"shift{m}")
                    nc.vector.tensor_mul(shift[:], mv[:, 0:1], rstd[:])
                    nc.vector.tensor_scalar_mul(shift[:], shift[:], -1.0)
                    for qh in range(4):
                        ot = out_pool.tile([P, D_OUT // 4], F32,
                                           name=f"ot{m}_{qh}", tag="ot")
                        nc.scalar.activation(
                            out=ot[:],
                            in_=zrow[:, qh * (D_OUT // 4):
                                     (qh + 1) * (D_OUT // 4)],
                            func=mybir.ActivationFunctionType.Relu,
                            bias=shift[:], scale=rstd[:],
                        )
                        for qq in range(NJ // 4):
                            qs = qh * (NJ // 4) + qq
                            rq = (pid_sc + qs) & 7
                            nc.scalar.dma_start(
                                out=out_ext[m * P:(m + 1) * P,
                                            bass.ds(rq * O_SHARD, O_SHARD)],
                                in_=ot[:, qq * O_SHARD:(qq + 1) * O_SHARD])

    nc.finalize()
    return nc


def kernel(x: np.ndarray, weight: np.ndarray, b: np.ndarray) -> np.ndarray:
    global last_exec_time_ns
    import os
    x = np.ascontiguousarray(x, dtype=np.float32)
    weight = np.ascontiguousarray(weight, dtype=np.float32)
    b = np.ascontiguousarray(b, dtype=np.float32)
    assert x.shape == (T_FULL, D_IN) and weight.shape == (D_OUT, D_IN)

    if "nc" not in _cache:
        _cache["nc"] = _build()
    nc = _cache["nc"]

    in_maps = [
        {
            "x": x[c * T_SHARD:(c + 1) * T_SHARD],
            "w": weight[c * O_SHARD:(c + 1) * O_SHARD],
            "b": b,
        }
        for c in range(N_CORES)
    ]
    trace = os.environ.get("BASS_KERNEL_TRACE", "") == "1"
    res = run_bass_kernel_spmd(nc, in_maps, list(range(N_CORES)), trace=trace)
    last_exec_time_ns = res.exec_time_ns
    return np.concatenate([res.results[c]["out"] for c in range(N_CORES)],
                          axis=0)


# revision 5
# speedup vs baseline: 1.2897x; 1.1592x over previous
"""Trainium2 Bass kernel for nn_BinaryLinear (8-core SPMD) — v4.

z = x @ binarize(W).T + binarize(b); out = relu((z - mean)/(std + eps))

v4 over v3 (see v3 docstring for the DoubleRow hi/lo scheme):
  - Own-slice-first main loop: iteration jj processes o-block
    r = (pid + jj) & 7. jj=0 is the core's own block, fed straight from
    the resident transposed tile — it runs before/while the AllGather
    flies. z column-slots are permuted per core (layernorm stats are
    permutation-invariant); only the wg-load sources, the permuted bias
    loads, and the final out-store column offsets use register offsets.
  - AllGather split into 2x 1MB k-half stages (2MB single AG measured
    140us vs 51us per 1MB); main loop split into pass-A (k0..15, biased
    partial z in bf16) and pass-B (k16..31 added on top + stats), so
    pass-A hides AG1.
  - Batched transpose evictions: 4 transposes land in one [128,512] PSUM
    tile; one ACT copy (hi) + one DVE subtract (lo) — or one is_gt for
    the w side — evict 4 k-tiles at once (v3's per-tile evictions made
    the x pipeline latency-bound).
  - The w pass-1 mean-reduces are fenced before the x-path's first DVE
    op so the threshold collective triggers at ~30us.
"""
import numpy as np

import concourse.bass as bass
import concourse.mybir as mybir
import concourse.tile as tile
from concourse import bacc
from concourse.bass_utils import run_bass_kernel_spmd
from concourse.masks import make_identity

N_CORES = 8
T_FULL = 8192
D_IN = 4096
D_OUT = 4096
T_SHARD = T_FULL // N_CORES    # 1024
O_SHARD = D_OUT // N_CORES     # 512
P = 128
NK = D_IN // P                 # 32 k-tiles
NKH = NK // 2                  # 16 per AG stage
NM = T_SHARD // P              # 8 token tiles
NJ = D_OUT // O_SHARD          # 8 o-blocks
QW = D_IN // 4                 # 1024
EPS = 1e-5
F32 = mybir.dt.float32
BF16 = mybir.dt.bfloat16
FP8 = mybir.dt.float8e4
DR = mybir.MatmulPerfMode.DoubleRow

_cache: dict = {}
last_exec_time_ns = None


def _pair0(ap2d):
    return bass.AP(tensor=ap2d.tensor, offset=ap2d.offset,
                   ap=[ap2d.ap[0], [0, 2], ap2d.ap[-1]])


def _bcast(ap, n_part, width=1):
    return bass.AP(tensor=ap.tensor, offset=ap.offset,
                   ap=[[0, n_part], [1, width]])


def _build():
    nc = bacc.Bacc("TRN2", target_bir_lowering=False, debug=False,
                   num_devices=N_CORES)
    x_in = nc.dram_tensor("x", [T_SHARD, D_IN], F32, kind="ExternalInput")
    w_in = nc.dram_tensor("w", [O_SHARD, D_IN], F32, kind="ExternalInput")
    b_in = nc.dram_tensor("b", [D_OUT], F32, kind="ExternalInput")
    out_ext = nc.dram_tensor("out", [T_SHARD, D_OUT], F32, kind="ExternalOutput")

    with tile.TileContext(nc) as tc:
        with (
            tc.tile_pool(name="xT_pool", bufs=1) as xT_pool,
            tc.tile_pool(name="z_pool", bufs=1) as z_pool,
            tc.tile_pool(name="wtr_pool", bufs=1) as wtr_pool,
            tc.tile_pool(name="wg_pool", bufs=2) as wg_pool,
            tc.tile_pool(name="wstage", bufs=2) as wstage,
            tc.tile_pool(name="wqb_pool", bufs=1) as wqb_pool,
            tc.tile_pool(name="xbf_pool", bufs=2) as xbf_pool,
            tc.tile_pool(name="out_pool", bufs=2) as out_pool,
            tc.tile_pool(name="bias_pool", bufs=1) as bias_pool,
            tc.tile_pool(name="small", bufs=1) as small,
            tc.tile_pool(name="psum", bufs=3, space="PSUM") as psum,
            tc.tile_pool(name="ptr_w", bufs=2, space="PSUM") as ptr_w,
            tc.tile_pool(name="ptr_x", bufs=2, space="PSUM") as ptr_x,
            tc.tile_pool(name="dram", bufs=1, space="DRAM") as dram,
        ):
            # xT pairs: ksub 2k = hi(e4m3), 2k+1 = lo; cols m*128..(m+1)*128
            xT = xT_pool.tile([P, 2 * NK, T_SHARD], FP8)               # 8 MB
            z_sb = [z_pool.tile([P, D_OUT], BF16, name=f"z{m}")        # 8 MB
                    for m in range(NM)]
            wtr = wtr_pool.tile([P, NK, O_SHARD], FP8)                 # 2 MB

            identity = small.tile([P, P], BF16)
            make_identity(nc, identity)

            # ---- A: w pass-1: load f32 chunks, partial sums -> threshold ----
            colsums = small.tile([P, 8], F32)
            w_first_dma = None
            last_reduce = None
            for ch in range(8):
                c4, hh = ch // 2, ch % 2
                wf = wstage.tile([P, 2 * QW], F32, name=f"wf{ch}", tag="ws")
                wdma = nc.sync.dma_start(
                    out=wf[:],
                    in_=w_in[c4 * P:(c4 + 1) * P,
                             hh * 2 * QW:(hh + 1) * 2 * QW])
                if w_first_dma is None:
                    w_first_dma = wdma
                w_last_p1_dma = wdma
                last_reduce = nc.vector.reduce_sum(
                    colsums[:, ch:ch + 1], wf[:], axis=mybir.AxisListType.X)
            rowsum = small.tile([P, 1], F32)
            nc.vector.reduce_sum(rowsum[:], colsums[:], axis=mybir.AxisListType.X)
            rs_dram = dram.tile([P], F32)
            nc.sync.dma_start(
                out=rs_dram[:].rearrange("(p o) -> p o", p=P), in_=rowsum[:])
            rs_row = small.tile([1, P], F32)
            nc.sync.dma_start(
                out=rs_row[:], in_=rs_dram[:].rearrange("(o p) -> o p", o=1))
            ar_sb = small.tile([1, 8], F32)
            nc.vector.memset(ar_sb[:], 0.0)
            nc.vector.reduce_sum(ar_sb[:, 0:1], rs_row[:],
                                 axis=mybir.AxisListType.X)
            ar_in = dram.tile([8], F32)
            ar_out = dram.tile([8], F32, addr_space="Shared")
            nc.sync.dma_start(out=ar_in[:].rearrange("(o d) -> o d", o=1),
                              in_=ar_sb[:])
            nc.gpsimd.collective_compute(
                "AllReduce", mybir.AluOpType.add,
                replica_groups=[list(range(N_CORES))],
                ins=[ar_in.opt()], outs=[ar_out.opt()],
            )
            thr_sb = small.tile([P, 1], F32)
            nc.gpsimd.dma_start(out=thr_sb[:], in_=_bcast(ar_out.opt(), P))
            nc.vector.tensor_scalar_mul(thr_sb[:], thr_sb[:],
                                        1.0 / (D_OUT * D_IN))

            # ---- B/C: w pass-2 per k-half: reload, cast, binarize,
            #           transpose (batched evictions), store + AllGather ----
            ag_in = [dram.tile([P * NKH * O_SHARD], FP8, name=f"agi{h}")
                     for h in range(2)]
            ag_out = [dram.tile([N_CORES, P * NKH * O_SHARD], FP8,
                                name=f"ago{h}", addr_space="Shared")
                      for h in range(2)]
            for h in range(2):
                for c4 in range(4):
                    wq = wqb_pool.tile([P, 2 * QW], BF16,
                                       name=f"wq{c4}_{h}", tag="wqb")
                    wf = wstage.tile([P, 2 * QW], F32, name=f"wf2_{c4}_{h}",
                                     tag="ws")
                    nc.sync.dma_start(
                        out=wf[:],
                        in_=w_in[c4 * P:(c4 + 1) * P,
                                 h * 2 * QW:(h + 1) * 2 * QW])
                    nc.vector.tensor_scalar(
                        out=wq[:], in0=wf[:],
                        scalar1=thr_sb[:], scalar2=None,
                        op0=mybir.AluOpType.is_gt,
                    )
                    for g in range(4):          # groups of 4 k-tiles
                        pt = ptr_w.tile([P, 4 * P], BF16,
                                        name=f"wpt{h}_{c4}_{g}", tag="ptrw")
                        for i in range(4):
                            nc.tensor.transpose(
                                pt[:, i * P:(i + 1) * P],
                                wq[:, (g * 4 + i) * P:(g * 4 + i + 1) * P],
                                identity[:])
                        k0 = h * NKH + g * 4
                        nc.vector.tensor_copy(
                            out=wtr[:, k0:k0 + 4, c4 * P:(c4 + 1) * P],
                            in_=pt[:])
                nc.sync.dma_start(
                    out=ag_in[h][:].rearrange("(p k o) -> p k o", p=P, k=NKH),
                    in_=wtr[:, h * NKH:(h + 1) * NKH, :])
                nc.gpsimd.collective_compute(
                    "AllGather", mybir.AluOpType.bypass,
                    replica_groups=[list(range(N_CORES))],
                    ins=[ag_in[h].opt()], outs=[ag_out[h].opt()],
                )

            # ---- D: bias binarize ([128, 32] layout) + permuted broadcast ----
            bsb = small.tile([P, D_OUT // P], F32)
            nc.scalar.dma_start(
                out=bsb[:], in_=b_in[:].rearrange("(p j) -> p j", p=P))
            bpart = small.tile([P, 1], F32)
            nc.vector.reduce_sum(bpart[:], bsb[:], axis=mybir.AxisListType.X)
            bp_dram = dram.tile([P], F32)
            nc.scalar.dma_start(
                out=bp_dram[:].rearrange("(p o) -> p o", p=P), in_=bpart[:])
            nc.scalar.dma_start(
                out=rs_row[:], in_=bp_dram[:].rearrange("(o p) -> o p", o=1))
            b_mean1 = small.tile([1, 1], F32)
            nc.vector.reduce_sum(b_mean1[:], rs_row[:], axis=mybir.AxisListType.X)
            nc.vector.tensor_scalar_mul(b_mean1[:], b_mean1[:], 1.0 / D_OUT)
            bm_dram = dram.tile([1], F32)
            nc.gpsimd.dma_start(out=bm_dram[:].rearrange("(o d) -> o d", o=1),
                                in_=b_mean1[:])
            b_mean = small.tile([P, 1], F32)
            nc.gpsimd.dma_start(out=b_mean[:], in_=_bcast(bm_dram.opt(), P))
            b_q128 = small.tile([P, D_OUT // P], FP8)
            nc.vector.tensor_scalar(
                out=b_q128[:], in0=bsb[:], scalar1=b_mean[:], scalar2=None,
                op0=mybir.AluOpType.is_gt,
            )
            bq_dram = dram.tile([D_OUT], FP8)
            nc.gpsimd.dma_start(
                out=bq_dram[:].rearrange("(p j) -> p j", p=P), in_=b_q128[:])
            bias_bcast = bias_pool.tile([P, D_OUT], FP8)                # 0.5 MB
            nc.gpsimd.dma_start(
                out=bias_bcast[:],
                in_=bass.AP(tensor=bq_dram.opt().tensor, offset=0,
                            ap=[[0, P], [1, D_OUT]]))

            # ---- E: x load/cast/transpose, batched hi/lo pair split ----
            xq_first = None
            first_lo = None
            for mx in range(NM):
                for xh in range(2):
                    xbf = xbf_pool.tile([P, 2 * QW], BF16,
                                        name=f"xbf{mx}_{xh}", tag="xbf")
                    xd = nc.gpsimd.dma_start(
                        out=xbf[:],
                        in_=x_in[mx * P:(mx + 1) * P,
                                 xh * 2 * QW:(xh + 1) * 2 * QW])
                    if xq_first is None:
                        xq_first = xd
                        tile.add_dep_helper(
                            w_last_p1_dma.ins, xd.ins, sync=True,
                            reason="w pass-1 loads win HBM first")
                    for q in range(4):
                        # one [128, 512] psum group = 4 transposed k-tiles
                        pt = ptr_x.tile([P, 4 * P], BF16,
                                        name=f"xpt{mx}_{xh}_{q}", tag="ptrx")
                        for i in range(4):
                            nc.tensor.transpose(
                                pt[:, i * P:(i + 1) * P],
                                xbf[:, (q * 4 + i) * P:(q * 4 + i + 1) * P],
                                identity[:])
                        k0 = xh * NKH + q * 4
                        xap = xT[:]
                        hi = bass.AP(tensor=xap.tensor,
                                     offset=(2 * k0) * T_SHARD + mx * P,
                                     ap=[xap.ap[0], [2 * T_SHARD, 4], [1, P]])
                        lo = bass.AP(tensor=xap.tensor,
                                     offset=(2 * k0 + 1) * T_SHARD + mx * P,
                                     ap=[xap.ap[0], [2 * T_SHARD, 4], [1, P]])
                        nc.scalar.copy(out=hi, in_=pt[:])
                        lo_tt = nc.vector.tensor_tensor(
                            out=lo, in0=pt[:], in1=hi,
                            op=mybir.AluOpType.subtract)
                        if first_lo is None:
                            first_lo = lo_tt
                            tile.add_dep_helper(
                                last_reduce.ins, lo_tt.ins, sync=True,
                                reason="threshold reduces first on DVE")

            # ---- F: main loop; slot jj <-> o-block (pid+jj)&7 ----
            stats = [small.tile([P, NJ, 6], F32, name=f"stats{m}")
                     for m in range(NM)]
            pid_s = nc.sync.partition_id()

            def wg_load(h, jj, tag_name):
                """Load o-block (pid+jj)&7 of AG stage h into SBUF."""
                wg = wg_pool.tile([P, NKH, O_SHARD], FP8, name=tag_name,
                                  tag="wg")
                r = (pid_s + jj) & 7
                src = bass.AP(
                    tensor=ag_out[h].opt().tensor,
                    offset=r * (P * NKH * O_SHARD),
                    ap=[[NKH * O_SHARD, P], [O_SHARD, NKH], [1, O_SHARD]])
                nc.sync.dma_start(out=wg[:], in_=src)
                return wg

            def mm_group(ps, wg_ap_fn, m, ks, start, stop):
                for i, k in enumerate(ks):
                    nc.tensor.matmul(
                        ps[:],
                        xT[:, 2 * k:2 * k + 2, m * P:(m + 1) * P],
                        _pair0(wg_ap_fn(k)),
                        start=(start and i == 0),
                        stop=(stop and i == len(ks) - 1),
                        perf_mode=DR)

            pid_v = nc.vector.partition_id()
            # pass-A: jj=0 full-k from resident wtr; jj>=1 k-half 0
            for jj in range(NJ):
                if jj == 0:
                    wg_ap_fn = lambda k: wtr[:, k, :]
                    ks = list(range(NK))
                else:
                    wg = wg_load(0, jj, f"wgA{jj}")
                    wg_ap_fn = (lambda wg: lambda k: wg[:, k, :])(wg)
                    ks = list(range(NKH))
                rv = ((pid_v + jj) & 7) * O_SHARD
                for m in range(NM):
                    ps = psum.tile([P, O_SHARD], F32, name=f"psA{jj}_{m}",
                                   tag="ps")
                    mm_group(ps, wg_ap_fn, m, ks, True, True)
                    zrow = z_sb[m]
                    nc.vector.tensor_tensor(
                        out=zrow[:, bass.ds(rv, O_SHARD)],
                        in0=ps[:],
                        in1=bias_bcast[:, bass.ds(rv, O_SHARD)],
                        op=mybir.AluOpType.add)
                    if jj == 0:
                        nc.vector.bn_stats(
                            out=stats[m][:, 0, :],
                            in_=zrow[:, bass.ds(rv, O_SHARD)])

            # pass-B: jj-outer (one 1MB wg load per jj keeps DMA light so
            # the PE stays at its fast clock); z written at true block
            # offsets so the normalize stores are big and static.
            for jj in range(1, NJ):
                wg = wg_load(1, jj, f"wgB{jj}")
                wg_ap_fn = (lambda wg: lambda k: wg[:, k - NKH, :])(wg)
                rv = ((pid_v + jj) & 7) * O_SHARD
                for m in range(NM):
                    ps = psum.tile([P, O_SHARD], F32, name=f"psB{jj}_{m}",
                                   tag="ps")
                    mm_group(ps, wg_ap_fn, m, list(range(NKH, NK)), True, True)
                    zrow = z_sb[m]
                    nc.vector.tensor_tensor(
                        out=zrow[:, bass.ds(rv, O_SHARD)],
                        in0=ps[:],
                        in1=zrow[:, bass.ds(rv, O_SHARD)],
                        op=mybir.AluOpType.add)
                    nc.vector.bn_stats(
                        out=stats[m][:, jj, :],
                        in_=zrow[:, bass.ds(rv, O_SHARD)])
                    if jj == NJ - 1:
                        mv = small.tile([P, 2], F32, name=f"mv{m}")
                        nc.vector.bn_aggr(out=mv[:], in_=stats[m][:])
                        std = small.tile([P, 1], F32, name=f"std{m}")
                        nc.scalar.sqrt(std[:], mv[:, 1:2])
                        nc.vector.tensor_scalar_add(std[:], std[:], EPS)
                        rstd = small.tile([P, 1], F32, name=f"rstd{m}")
                        nc.vector.reciprocal(rstd[:], std[:])
                        shift = small.tile([P, 1], F32, name=f"shift{m}")
                        nc.vector.tensor_mul(shift[:], mv[:, 0:1], rstd[:])
                        nc.vector.tensor_scalar_mul(shift[:], shift[:], -1.0)
                        store_qs = [nc.sync, nc.scalar, nc.gpsimd]
                        for qh in range(4):
                            ot = out_pool.tile([P, QW], F32,
                                               name=f"ot{m}_{qh}", tag="ot")
                            nc.scalar.activation(
                                out=ot[:],
                                in_=zrow[:, qh * QW:(qh + 1) * QW],
                                func=mybir.ActivationFunctionType.Relu,
                                bias=shift[:], scale=rstd[:],
                            )
                            store_qs[(m * 4 + qh) % 3].dma_start(
                                out=out_ext[m * P:(m + 1) * P,
                                            qh * QW:(qh + 1) * QW],
                                in_=ot[:])

    nc.finalize()
    return nc


def kernel(x: np.ndarray, weight: np.ndarray, b: np.ndarray) -> np.ndarray:
    global last_exec_time_ns
    import os
    x = np.ascontiguousarray(x, dtype=np.float32)
    weight = np.ascontiguousarray(weight, dtype=np.float32)
    b = np.ascontiguousarray(b, dtype=np.float32)
    assert x.shape == (T_FULL, D_IN) and weight.shape == (D_OUT, D_IN)

    if "nc" not in _cache:
        _cache["nc"] = _build()
    nc = _cache["nc"]

    in_maps = [
        {
            "x": x[c * T_SHARD:(c + 1) * T_SHARD],
            "w": weight[c * O_SHARD:(c + 1) * O_SHARD],
            "b": b,
        }
        for c in range(N_CORES)
    ]
    trace = os.environ.get("BASS_KERNEL_TRACE", "") == "1"
    res = run_bass_kernel_spmd(nc, in_maps, list(range(N_CORES)), trace=trace)
    last_exec_time_ns = res.exec_time_ns
    return np.concatenate([res.results[c]["out"] for c in range(N_CORES)],
                          axis=0)
